# revision 108
# baseline (speedup 1.0000x reference)
"""DiffAttn3d Trainium2 kernel.

8-core sharding: core c -> (batch b = c//4, query slice qs = (c%4)*512).
Each core computes its 512-query slice of the full differential-attention
block (all 16 n-heads) and the final output projection for that slice.

Flash-style with scores computed transposed (keys on partitions, queries on
the free axis).  Per core the key axis is ROTATED so this core's queries are
always rows 0..511 of xs (attention is permutation-invariant over keys when
mask and values rotate with them) - queries are then just the first 512
columns of xsT and need no separate load/transpose.

Score-PSUM evacuation (the serial bottleneck: 16.8M exp's) is split across
two engines per score unit: ~19/32 j-units drain on ACT via exp, ~13/32 on
the DVE via a one-instruction bf16 Schraudolph exp bit trick
(bf16_bits = int16(s*A + M[k,q])).  For ACT units the boolean mask is
preloaded into the score PSUM by an fp8 DoubleRow matmul (I.T @ (60*m) at
half cycle cost, host-packed fp8) plus a -60 exp bias; for DVE units the
mask rides in the per-element int16 Schraudolph bias tensor M (16251
unmasked / 5171 masked, both int16-exact), so those units need no mask
matmul at all.  Masked entries become exp(s-60) ~ 1e-26 ~ 0.  Score/AV
matmul operands are bf16; accumulation stays fp32 in PSUM.

The AV matmul uses e (exp scores) as the STATIONARY operand so its output
lands directly in natural [query, head-dim] layout with no transposes:
av[q,d] = sum_k e[k,q] * v[k,d]; the softmax denominator rides along as a
ones-column appended to v.  Because PSUM allows only one accumulation group
per 2KB bank, the AV pass for pair g runs one iteration behind the score
pass (e tiles are buffered for a full pair) with its 8 (j,qs) groups issued
sequentially into 2 alternating banks, entries spread every 4th score unit
(offset 1, so the prior pair's last drain clears before the first group
reads its e tile).  Both per-j score rings are 3 deep; the projection tile
lives in the sps0 ring (it is only live at iteration end) and the
projection epilogue for pair g-2 transposes through a bf16 view of that
not-yet-written bank.  RMSNorm uses a fast inverse-sqrt (bit trick + 1
Newton step) on the DVE; the differential combine is computed
scale-invariantly (at = av0 - lam*(den0/den1)*av1) and the 1/rms scale is
applied after the output projection (both commute).
"""

import math
import numpy as np

B, L, IN_DIM, OUT_DIM = 2, 2048, 128, 128
H, DH = 8, 32
ED = H * DH * 2          # 512
NH = 2 * H               # 16 n-heads
DEPTH = 1
LAMBDA_INIT = 0.8 - 0.6 * math.exp(-0.3 * (DEPTH + 1))
EPS = 1e-8

QSL = 512                # queries per core
NKC = L // 128           # 16 key chunks
NSC = L // 512           # 4 seq chunks of 512
NQS = QSL // 128         # 4 query subtiles
NCH = 6                  # head chunks: 3 heads per 128 partitions (base
                         # partition of matmul operands must be 0/32/64)
MASK_BIG = 60.0
# bf16 Schraudolph exp: bits(e) ~ int16(s*A + B); B folds the -60 mask bias
SCH_A = 128 * math.log2(math.e)
SCH_B = 16256.0 - 5.25 - SCH_A * MASK_BIG
# per-element drain bias (int16-exact): unmasked ~ 16256-5, masked folds -60
SCH_M1 = 16251
SCH_M0 = 16251 - round(SCH_A * MASK_BIG)

_CACHE = {}


def _build_program():
    import concourse.bass as bass
    import concourse.tile as tile
    from concourse import bacc, mybir
    from concourse.masks import make_identity

    f32 = mybir.dt.float32
    bf16 = mybir.dt.bfloat16
    fp8 = mybir.dt.float8e4
    i16 = mybir.dt.int16
    u32 = mybir.dt.uint32
    AF = mybir.ActivationFunctionType
    ALU = mybir.AluOpType
    DR = mybir.MatmulPerfMode.DoubleRow

    nc = bacc.Bacc("TRN2", target_bir_lowering=False, debug=False,
                   num_devices=8)

    xs_d = nc.declare_dram_parameter("xs", [L, IN_DIM], f32, isOutput=False)
    # mask pre-packed for DoubleRow: [k, 2, q] fp8 with [:,0,:]=60*m, [:,1,:]=0
    mp_d = nc.declare_dram_parameter("maskp", [L, 2, QSL], fp8, isOutput=False)
    # int16 Schraudolph bias tensor: 16251 where unmasked, 5171 where masked
    mi_d = nc.declare_dram_parameter("maski", [L, QSL], i16, isOutput=False)
    # w = [Wq padded to 6 chunks | Wk padded to 6 chunks | Wv] (see host prep)
    w_d = nc.declare_dram_parameter("w", [IN_DIM, NCH * 256 + ED], f32,
                                    isOutput=False)
    wo_d = nc.declare_dram_parameter("wo", [ED, OUT_DIM], f32, isOutput=False)
    nlam_d = nc.declare_dram_parameter("nlam", [128, 1], f32, isOutput=False)
    out_d = nc.declare_dram_parameter("out", [QSL, OUT_DIM], f32, isOutput=True)

    with tile.TileContext(nc) as tc:
        with (
            tc.tile_pool(name="const", bufs=1) as const,
            tc.tile_pool(name="xin", bufs=1) as xin_p,
            tc.tile_pool(name="psA", bufs=2, space=bass.MemorySpace.PSUM) as psA,
            tc.tile_pool(name="avp", bufs=1, space=bass.MemorySpace.PSUM) as avp,
            tc.tile_pool(name="epool", bufs=45) as epool,
            tc.tile_pool(name="tinyp", bufs=2) as tinyp,
            tc.tile_pool(name="tmpp", bufs=2) as tmpp,
        ):
            # ---- constants / weights ----
            # x DMAs first (they head the dependency chain), then w in 3
            # slices (Wk first) so the kT projection starts early; the bf16
            # converts are spread across DVE/ACT/Pool
            xins = []
            for s4 in range(NSC):
                xin = xin_p.tile([128, 4, 128], f32, tag=f"xin{s4}",
                                 name="xin")
                nc.sync.dma_start(
                    xin[:], xs_d[s4 * 512:(s4 + 1) * 512, :].rearrange(
                        "(c p) d -> p c d", p=128))
                xins.append(xin)
            w_sb = const.tile([128, NCH * 256 + ED], f32)
            w_bf = const.tile([128, NCH * 256 + ED], bf16)
            wk_lo, wv_lo = NCH * 128, NCH * 256
            for eng, (lo, hi) in zip(
                    (nc.vector, nc.scalar, nc.gpsimd),
                    ((wk_lo, wv_lo), (0, wk_lo), (wv_lo, wv_lo + ED))):
                nc.sync.dma_start(w_sb[:, lo:hi], w_d[:, lo:hi])
                if eng is nc.scalar:
                    nc.scalar.copy(w_bf[:, lo:hi], w_sb[:, lo:hi])
                else:
                    eng.tensor_copy(w_bf[:, lo:hi], w_sb[:, lo:hi])
            # first mask chunks - g0's first score units need them early
            mk8 = const.tile([128, NKC, 2, QSL], fp8)
            mi = const.tile([128, NKC, QSL], i16)
            for pf in range(3):
                nc.sync.dma_start(mk8[:, pf, :, :],
                                  mp_d[pf * 128:(pf + 1) * 128, :, :])
                nc.sync.dma_start(mi[:, pf, :],
                                  mi_d[pf * 128:(pf + 1) * 128, :])
            wo_sb = const.tile([64, H, 128], f32)
            nc.sync.dma_start(wo_sb[:], wo_d.rearrange("(t p) o -> p t o", p=64))
            wo_bf = const.tile([64, H, 128], bf16)
            nc.gpsimd.tensor_copy(wo_bf[:], wo_sb[:])
            nlam_sb = const.tile([128, 1], f32)
            nc.sync.dma_start(nlam_sb[:], nlam_d[:])

            eye1 = const.tile([128, 128], f32)
            make_identity(nc, eye1[:])
            eyebf = const.tile([128, 128], bf16)
            nc.vector.tensor_copy(eyebf[:], eye1[:])
            # DoubleRow stationary: [:,0,:] = identity, [:,1,:] = 0 (fp8)
            w8 = const.tile([128, 2, 128], fp8)
            nc.gpsimd.memset(w8[:, 1, :], 0.0)
            nc.vector.tensor_copy(w8[:, 0, :], eye1[:])
            negbig = const.tile([128, 1], f32)
            nc.vector.memset(negbig[:], -MASK_BIG)
            magic = const.tile([128, NQS], u32)
            nc.vector.memset(magic[:], 0x5F3759DF)

            # ---- xsT: transpose x [L,128] -> [128, L] (bf16 out) ----
            xsT = const.tile([128, L], bf16)
            for s4 in range(NSC):
                ps = psA.tile([128, 512], f32, tag=f"sps{s4 % 2}",
                              name="ps", bufs=3)
                for t in range(4):
                    nc.tensor.transpose(ps[:, t * 128:(t + 1) * 128],
                                        xins[s4][:, t, :], eye1[:])
                if s4 % 2:
                    nc.scalar.copy(xsT[:, s4 * 512:(s4 + 1) * 512], ps[:])
                else:
                    nc.vector.tensor_copy(xsT[:, s4 * 512:(s4 + 1) * 512],
                                          ps[:])

            # ---- projections (bf16 in/out, fp32 psum) ----
            qT = const.tile([128, NCH, QSL], bf16)
            kT = const.tile([128, NCH, L], bf16)

            def emit_qproj(c):
                # qT[c] = (Wq_pad[:, c*128:+128]).T @ xsT[:, 0:QSL]
                ps = psA.tile([128, 512], f32, tag="sps0", name="ps",
                              bufs=3)
                nc.tensor.matmul(ps[:], w_bf[:, c * 128:(c + 1) * 128],
                                 xsT[:, 0:QSL], start=True, stop=True)
                nc.vector.tensor_copy(qT[:, c, :], ps[:])

            def emit_kproj(c, blocks):
                # kT[c] = (Wk_pad[:, c*128:+128]).T @ xsT
                for s in blocks:
                    ps = psA.tile([128, 512], f32, tag=f"sps{s % 2}",
                                  name="ps", bufs=3)
                    nc.tensor.matmul(
                        ps[:],
                        w_bf[:, NCH * 128 + c * 128:NCH * 128 + (c + 1) * 128],
                        xsT[:, s * 512:(s + 1) * 512], start=True, stop=True)
                    if s % 2:
                        nc.scalar.copy(kT[:, c, s * 512:(s + 1) * 512], ps[:])
                    else:
                        nc.vector.tensor_copy(kT[:, c, s * 512:(s + 1) * 512],
                                              ps[:])

            def emit_proj(c):
                emit_qproj(c)
                emit_kproj(c, range(NSC))

            # only the first kT block of chunks 0/1 is needed to start g0;
            # later blocks are emitted just-in-time inside g0's kc loop
            emit_qproj(0)
            emit_kproj(0, [0])
            emit_qproj(1)
            emit_kproj(1, [0])

            # v+ones and fp8 mask tiles are filled lazily in g==0's kc loop
            vp = const.tile([128, NKC, H, 65], bf16)
            nc.vector.memset(vp[:, :, :, 64:65], 1.0)

            out_acc = const.tile([128, NQS, 128], f32)

            # ---- attention: 8 score passes; AV + epilogue pipelined ----
            # p1: state of pair g-1 (e tiles -> AV this iter)
            # p2: state of pair g-2 (at/rr -> projection this iter)
            p1 = None
            p2 = None
            for g in range(H + 2):
                escore = []

                def emit_av_qs(qs):
                    # AV + DVE epilogue for pair g-1, query subtile qs
                    gp, ep = p1["g"], p1["e"]
                    avt = avp.tile([128, 2, 65], f32,
                                   tag=f"av{(p1['g'] * 4 + qs) % 2}",
                                   name="avt")
                    for j in range(2):
                        for kc in range(NKC):
                            nc.tensor.matmul(
                                avt[:, j, :],
                                ep[kc][:, j, qs * 128:(qs + 1) * 128],
                                vp[:, kc, gp, :],
                                start=(kc == 0), stop=(kc == NKC - 1))
                    # rms-normalization makes at scale-invariant per
                    # (pair, query), so scale through by den0:
                    # at = av0 - lam*(den0/den1)*av1
                    r1 = tinyp.tile([128, 1], f32, tag="r1", name="r1")
                    nc.vector.reciprocal(r1[:], avt[:, 1, 64:65])
                    rn = tinyp.tile([128, 1], f32, tag="rn", name="rn")
                    nc.vector.scalar_tensor_tensor(
                        rn[:], r1[:], nlam_sb[:], avt[:, 0, 64:65],
                        ALU.mult, ALU.mult)
                    t1 = tmpp.tile([128, 64], f32, tag="t1", name="t1")
                    nc.vector.tensor_scalar(t1[:], avt[:, 1, 0:64], rn[:],
                                            None, ALU.mult)
                    nc.vector.tensor_tensor(p1["at"][:, qs, :], t1[:],
                                            avt[:, 0, 0:64], ALU.add)
                    sqj = tmpp.tile([128, 64], bf16, tag="sqj", name="sqj")
                    nc.vector.scalar_tensor_tensor(
                        sqj[:], p1["at"][:, qs, :], 1.0, p1["at"][:, qs, :],
                        ALU.bypass, ALU.mult,
                        accum_out=p1["ss4"][:, qs:qs + 1])

                if p1 is not None:
                    p1["at"] = tmpp.tile([128, NQS, 64], bf16, tag="at",
                                         name="at")
                    p1["ss4"] = tinyp.tile([128, NQS], f32, tag="ss4",
                                           name="ss4")

                if g < H:
                    if g in (1, 2, 3, 4):
                        emit_proj(g + 1)
                    for kc in range(NKC):
                        if g == 0:
                            # lazily build v (+ones col); prefetch fp8 mask
                            # 3 chunks ahead; late kT blocks just-in-time
                            if kc + 3 < NKC:
                                nc.sync.dma_start(
                                    mk8[:, kc + 3, :, :],
                                    mp_d[(kc + 3) * 128:(kc + 4) * 128, :, :])
                                nc.sync.dma_start(
                                    mi[:, kc + 3, :],
                                    mi_d[(kc + 3) * 128:(kc + 4) * 128, :])
                            if kc in (1, 5, 9):
                                s = kc // 4 + 1
                                emit_kproj(0, [s])
                                emit_kproj(1, [s])
                            ps = psA.tile([128, 512], f32,
                                          tag=f"sps{kc % 2}", name="ps",
                                          bufs=3)
                            nc.tensor.matmul(
                                ps[:], xsT[:, kc * 128:(kc + 1) * 128],
                                w_bf[:, NCH * 256:NCH * 256 + ED],
                                start=True, stop=True)
                            if kc % 2:
                                nc.scalar.copy(
                                    vp[:, kc, :, 0:64],
                                    ps[:].rearrange("p (h d) -> p h d", h=H))
                            else:
                                nc.vector.tensor_copy(
                                    vp[:, kc, :, 0:64],
                                    ps[:].rearrange("p (h d) -> p h d", h=H))
                        e = epool.tile([128, 2, 512], bf16, tag="e", name="e")
                        for j in range(2):
                            n = 2 * g + j
                            c, r = n // 3, (n % 3) * 32
                            spsj = psA.tile([128, 512], f32, tag=f"sps{j}",
                                            name="spsj", bufs=3)
                            # static drain split: j0 on ACT, j1 mostly on the
                            # DVE so both engines evacuate score PSUM in
                            # parallel and the PE never stalls
                            act_drain = j == 0 or kc % 5 == 2
                            if act_drain:
                                # mask preload (+60*m) only for ACT units;
                                # DVE units get the mask via the per-element
                                # Schraudolph bias below
                                nc.tensor.matmul(spsj[:], w8[:],
                                                 mk8[:, kc, :, :],
                                                 start=True, stop=False,
                                                 perf_mode=DR)
                            nc.tensor.matmul(
                                spsj[:],
                                kT[r:r + 32, c, kc * 128:(kc + 1) * 128],
                                qT[r:r + 32, c, :], start=not act_drain,
                                stop=True)
                            if act_drain:
                                nc.scalar.activation(e[:, j, :], spsj[:],
                                                     AF.Exp, bias=negbig[:])
                            else:
                                nc.vector.scalar_tensor_tensor(
                                    e[:, j, :].bitcast(i16), spsj[:], SCH_A,
                                    mi[:, kc, :], ALU.mult, ALU.add)
                        escore.append(e)
                        # interleave prior pair's AV/epilogue between the
                        # first score units so its DVE ops sit ahead of most
                        # of this pair's drains in the in-order DVE queue
                        if p1 is not None and kc % 4 == 1:
                            emit_av_qs(kc // 4)
                elif p1 is not None:
                    for qs in range(NQS):
                        emit_av_qs(qs)

                if p1 is not None:
                    # rr = 1/sqrt(ss4/64): fast inverse sqrt, all on DVE
                    ss4 = p1["ss4"]
                    msx = tinyp.tile([128, NQS], f32, tag="msx", name="msx")
                    nc.vector.tensor_scalar(msx[:], ss4[:], 1.0 / 64, None,
                                            ALU.mult)
                    sh = tinyp.tile([128, NQS], u32, tag="sh", name="sh")
                    nc.vector.tensor_scalar(sh[:], msx[:].bitcast(u32), 1,
                                            None, ALU.logical_shift_right)
                    rr4 = tinyp.tile([128, NQS], f32, tag="rr4", name="rr4")
                    nc.vector.tensor_tensor(rr4[:].bitcast(u32), magic[:],
                                            sh[:], ALU.subtract)
                    nwu = tinyp.tile([128, NQS], f32, tag="nwu", name="nwu")
                    nww = tinyp.tile([128, NQS], f32, tag="nww", name="nww")
                    for _ in range(1):
                        nc.vector.tensor_tensor(nwu[:], rr4[:], rr4[:],
                                                ALU.mult)
                        nc.vector.scalar_tensor_tensor(
                            nwu[:], nwu[:], 0.5, msx[:], ALU.mult, ALU.mult)
                        nc.vector.tensor_scalar(nww[:], nwu[:], -1.0, 1.5,
                                                ALU.mult, ALU.add)
                        nc.vector.tensor_tensor(rr4[:], rr4[:], nww[:],
                                                ALU.mult)
                    p1["rr4"] = rr4

                if p2 is not None:
                    # PE epilogue for pair g-2: transpose, project, scale
                    gp = p2["g"]
                    at, rr4 = p2["at"], p2["rr4"]
                    op = psA.tile([128, NQS, 128], f32, tag="sps0",
                                  name="op", bufs=3)
                    atT = tmpp.tile([64, NQS, 128], bf16, tag="atT",
                                    name="atT")
                    # transpose into the (not yet written) op bank through a
                    # bf16 view; atT is copied out before the projection
                    # matmuls below overwrite the bank
                    tr4 = op[0:64, :, 0:64].bitcast(bf16)
                    for q in range(NQS):
                        nc.tensor.transpose(tr4[:, q, :], at[:, q, :],
                                            eyebf[:])
                    nc.scalar.copy(atT[:], tr4[:])
                    for q in range(NQS):
                        nc.tensor.matmul(op[:, q, :], atT[:, q, :],
                                         wo_bf[:, gp, :], start=True,
                                         stop=True)
                        # out_acc += rr * op  (1/rms commutes with the matmul)
                        if gp == 0:
                            nc.vector.tensor_scalar(out_acc[:, q, :],
                                                    op[:, q, :],
                                                    rr4[:, q:q + 1], None,
                                                    ALU.mult)
                        else:
                            nc.vector.scalar_tensor_tensor(
                                out_acc[:, q, :], op[:, q, :],
                                rr4[:, q:q + 1], out_acc[:, q, :],
                                ALU.mult, ALU.add)
                        if gp == H - 1:
                            # stream the finished query subtile out now;
                            # alternate the issuing sequencer so the final
                            # DMA completions overlap instead of serializing
                            deng = nc.scalar if q % 2 else nc.sync
                            deng.dma_start(
                                out_d.rearrange("(s p) o -> p s o",
                                                p=128)[:, q, :],
                                out_acc[:, q, :])

                p2 = p1
                p1 = {"g": g, "e": escore} if g < H else None


    nc.compile()
    return nc


def kernel(**inputs):
    import ml_dtypes
    from concourse.bass_utils import run_bass_kernel_spmd

    x = np.asarray(inputs["x"], np.float32)
    mask = np.asarray(inputs["mask_2d"]).astype(np.float32)
    Wq = np.asarray(inputs["Wq"], np.float32)
    Wkv = np.asarray(inputs["Wkv"], np.float32)
    Wout = np.asarray(inputs["Wout"], np.float32)
    lq1 = np.asarray(inputs["lambda_q1"], np.float32)
    lk1 = np.asarray(inputs["lambda_k1"], np.float32)
    lq2 = np.asarray(inputs["lambda_q2"], np.float32)
    lk2 = np.asarray(inputs["lambda_k2"], np.float32)
    gamma = np.asarray(inputs["gamma"], np.float32)

    lam = float(np.exp(np.sum(lq1 * lk1)) - np.exp(np.sum(lq2 * lk2))
                + LAMBDA_INIT)
    Wq_s = (Wq * DH ** -0.5).astype(np.float32)
    Wk = Wkv[:, :ED]
    Wv = Wkv[:, ED:]

    def pad_heads(Wm):
        # chunk c (128 cols) holds heads 3c..3c+2 at col offsets 0/32/64
        out = np.zeros((IN_DIM, NCH * 128), np.float32)
        for n in range(NH):
            c, r = divmod(n, 3)
            out[:, c * 128 + r * 32:c * 128 + r * 32 + 32] = \
                Wm[:, n * DH:(n + 1) * DH]
        return out

    W = np.ascontiguousarray(
        np.concatenate([pad_heads(Wq_s), pad_heads(Wk), Wv], axis=1))
    gs = (gamma * (1.0 - LAMBDA_INIT)).astype(np.float32)
    Wog = np.ascontiguousarray(Wout * np.tile(gs, H)[:, None])
    nlam = np.full((128, 1), -lam, np.float32)

    if "nc" not in _CACHE:
        _CACHE["nc"] = _build_program()
    nc = _CACHE["nc"]

    in_maps = []
    for c in range(8):
        b, qc = divmod(c, 4)
        # rotate keys so this core's queries are rows 0..511; attention is
        # permutation-invariant over keys when mask/values rotate with them
        xs_rot = np.roll(x[b, 0], -qc * QSL, axis=0)
        mT = np.roll(mask[b].T[:, qc * QSL:(qc + 1) * QSL], -qc * QSL, axis=0)
        mp = np.zeros((L, 2, QSL), ml_dtypes.float8_e4m3)
        mp[:, 0, :] = (MASK_BIG * mT).astype(ml_dtypes.float8_e4m3)
        mi = np.where(mT > 0, SCH_M1, SCH_M0).astype(np.int16)
        in_maps.append({
            "xs": np.ascontiguousarray(xs_rot),
            "maskp": mp,
            "maski": np.ascontiguousarray(mi),
            "w": W,
            "wo": Wog,
            "nlam": nlam,
        })

    r = run_bass_kernel_spmd(nc, in_maps, list(range(8)))
    _CACHE["last_results"] = r
    res = r.results

    out = np.empty((B, 1, L, OUT_DIM), np.float32)
    for c in range(8):
        b, qc = divmod(c, 4)
        out[b, 0, qc * QSL:(qc + 1) * QSL, :] = res[c]["out"]
    return out
